# revision 2
# baseline (speedup 1.0000x reference)
"""Paged-KV-cache causal GQA attention on 8 TRN2 NeuronCores.

Problem shape (hardcoded): B=8 seqs x S=1024 tokens, H=32 q-heads,
KVH=8 kv-heads (GQA group 4), D=128, block_size=256, 40 cache blocks.

Sharding: data parallel, one sequence per core. Host does the
store_kvcache scatter + block-table gather (layout work) and per-core
layout prep (head-major transposes + bf16 cast, scale folded into q);
each core runs causal flash attention for its sequence over all 32
heads. Softmax denominator division happens on the host: the device
ships unnormalized PV output plus the rowsum column.

Device algorithm per head, two heads interleaved, 6 score groups per
head (each <=1024 psum cols = 2 banks; pspool 3-deep so QK never waits
on exp):
  warmup: 28 dummy matmuls bridge the initial DMA wait so the PE HAM
          clock gate stays 8/8 into the first real matmul (~15us).
  QK:     scores^T[k,q] = K^T.T @ Q^T (PE, bf16) into a [128,<=1024]
          psum tile. Diagonal k-tiles are split into (diag 128) +
          (rest) matmuls sharing one weight load so the two diagonal
          subtiles of a group land contiguously at psum cols [0,256).
  exp:    one wide op per (head, group). The two full-width qc1 groups
          (kt0-3) of BOTH heads run a bf16-Schraudolph exp on the DVE
          (bits = rne(s*184.664 + 16248.6) as int16, bitcast bf16);
          the rest run on ACT. Load-balances ACT ~111us / DVE ~117us,
          both under the PE's ~135us.
  mask:   both diagonal subtiles of a group in ONE DVE multiply
          against a broadcast upper-tri mask.
  PV:     po[q,0:129] += P^T.T @ [V|1] per (q-tile, k-tile), two
          q-tiles per psum bank, emitted one group late so QK + exp
          stay ahead on the PE stream. No reciprocal / normalize:
          ACT copies the raw [128,258] po pair (128 out cols + rowsum
          col per q-tile) to SBUF as bf16; one store per head on the
          sync HWDGE ring (scalar-ring stores would queue behind exps).
Last unit runs PVs eagerly and pre-accumulates qt6/qt7 over kt0-5
during the final groups so only 3 matmuls + 2 copies trail the last
exp.
"""

import sys

import numpy as np
import ml_dtypes

sys.path.insert(0, "/opt/trn_rl_repo")

import concourse.bass as bass  # noqa: E402
import concourse.mybir as mybir  # noqa: E402
import concourse.tile as tile  # noqa: E402
from concourse import bacc  # noqa: E402
from concourse.bass_utils import run_bass_kernel_spmd  # noqa: E402

B, S = 8, 1024
H, KVH, D = 32, 8, 128
G = H // KVH
NT = S // 128  # 8 k/q tiles of 128 per sequence
VW = 132  # v tile row: 128 v cols + ones col + pad
SCALE = 1.0 / float(np.sqrt(D))
BF = mybir.dt.bfloat16
F32 = mybir.dt.float32
I16 = mybir.dt.int16
_NC = None

# Score groups: (qc, tw, entries) with entries = ordered matmuls
# (kt, q_tile_off_in_chunk, width, psum_off, is_diag). Diagonal k-tiles
# are split so the (up to two) diagonal subtiles of a group sit at psum
# cols [0,256); no region crosses a 512-f32 psum bank boundary. Entries
# for one kt are consecutive (one weight load each).
GROUPS6 = [
    (0, 896, [(0, 0, 128, 0, True), (0, 1, 384, 512, False),
              (1, 1, 128, 128, True), (1, 2, 256, 256, False)]),
    (0, 384, [(2, 2, 128, 0, True), (2, 3, 128, 256, False),
              (3, 3, 128, 128, True)]),
    (1, 1024, [(0, 0, 512, 0, False), (1, 0, 512, 512, False)]),
    (1, 1024, [(2, 0, 512, 0, False), (3, 0, 512, 512, False)]),
    (1, 896, [(4, 0, 128, 0, True), (4, 1, 384, 512, False),
              (5, 1, 128, 128, True), (5, 2, 256, 256, False)]),
    (1, 384, [(6, 2, 128, 0, True), (6, 3, 128, 256, False),
              (7, 3, 128, 128, True)]),
]
# q-tile pairs emitted (one group late) after each group index
PV_AFTER = {0: (0, 1), 1: (2, 3), 4: (4, 5), 5: (6, 7)}
# groups whose exp runs as bf16-Schraudolph on the DVE: (gi, head-in-pair)
SCHRAU = {(2, 0), (2, 1), (3, 0), (3, 1)}


def _build_nc():
    nc = bacc.Bacc("TRN2", target_bir_lowering=False, debug=False, num_devices=8)
    qT = nc.dram_tensor("qT", [H, D, S], BF, kind="ExternalInput").ap()
    kT = nc.dram_tensor("kT", [KVH, D, S], BF, kind="ExternalInput").ap()
    v1 = nc.dram_tensor("v1", [KVH, NT, 128, VW], BF, kind="ExternalInput").ap()
    # per head: 4 q-tile pairs x [128 rows, 2*(128 out + rowsum)]
    out = nc.dram_tensor("out", [H, 4, 128, 258], BF, kind="ExternalOutput").ap()
    mask_np = np.triu(np.ones((128, 128), dtype=ml_dtypes.bfloat16))
    mask_dram = nc.inline_tensor(mask_np, "tri_mask").ap()

    with tile.TileContext(nc) as tc:
        with (
            tc.tile_pool(name="singles", bufs=1) as singles,
            tc.tile_pool(name="qpool", bufs=6) as qpool,
            tc.tile_pool(name="ppool", bufs=16) as ppool,
            tc.tile_pool(name="dpool", bufs=14) as dpool,
            tc.tile_pool(name="opool", bufs=4) as opool,
            tc.tile_pool(name="pspool", bufs=3, space="PSUM") as pspool,
            tc.tile_pool(name="popool", bufs=2, space="PSUM") as popool,
        ):
            # --- HAM warmup: dummy matmuls with no data deps keep the
            # PE busy until the first real QK data lands (~15us) so the
            # clock gate is 8/8 when real work starts ---
            warm_sb = singles.tile([128, 256], BF, name="warm_sb")
            nc.vector.memset(warm_sb, 0.0)
            dummy_ps = popool.tile([128, 258], F32, tag="po", name="dummy_ps")
            for i in range(16):
                nc.tensor.matmul(
                    dummy_ps[:, 0:256], lhsT=warm_sb[:, 0:128], rhs=warm_sb,
                    start=True, stop=True, skip_group_check=True,
                )
            for i in range(12):
                nc.tensor.matmul(
                    dummy_ps[:, 0:128], lhsT=warm_sb[:, 0:128],
                    rhs=warm_sb[:, 0:128],
                    start=True, stop=True, skip_group_check=True,
                )

            mask_sb = singles.tile([128, 128], BF)
            kv_sb = []
            for kvh in range(KVH):
                k_t = singles.tile([128, S], BF, name=f"kT_sb{kvh}", tag=f"kT{kvh}")
                v_t = singles.tile(
                    [128, NT * VW], BF, name=f"v1_sb{kvh}", tag=f"v1{kvh}"
                )
                kv_sb.append((k_t, v_t))

            def load_kv(kvh):
                # kT on the sync HWDGE ring; v1 on the gpsimd SWDGE ring so
                # the two streams' kickoffs and transfers run in parallel
                nc.sync.dma_start(out=kv_sb[kvh][0], in_=kT[kvh])
                nc.gpsimd.dma_start(
                    out=kv_sb[kvh][1].rearrange("p (t c) -> p t c", t=NT),
                    in_=v1[kvh].rearrange("t p c -> p t c"),
                )

            q_tiles = {}

            def load_q(h):
                if h < H and h not in q_tiles:
                    q_tiles[h] = qpool.tile([128, S], BF, tag="q", name=f"q_sb{h}")
                    nc.sync.dma_start(out=q_tiles[h], in_=qT[h])

            # fast start: the head phase is HBM-bandwidth-bound (all 8
            # cores burst-load at once), so critical bytes go on ONE ring
            # in strict need order (first group needs only kT cols 0:256);
            # v1[0] (needed ~2us later for the first PVs) rides the
            # otherwise-idle scalar HWDGE ring
            q_tiles[0] = qpool.tile([128, S], BF, tag="q", name="q_sb0")
            q_tiles[1] = qpool.tile([128, S], BF, tag="q", name="q_sb1")
            nc.sync.dma_start(out=kv_sb[0][0][:, 0:256], in_=kT[0][:, 0:256])
            nc.sync.dma_start(out=q_tiles[0][:, 0:512], in_=qT[0][:, 0:512])
            nc.sync.dma_start(out=mask_sb, in_=mask_dram)
            nc.sync.dma_start(out=kv_sb[0][0][:, 256:512], in_=kT[0][:, 256:512])
            nc.sync.dma_start(out=q_tiles[1][:, 0:512], in_=qT[1][:, 0:512])
            nc.sync.dma_start(out=q_tiles[0][:, 512:], in_=qT[0][:, 512:])
            nc.sync.dma_start(out=q_tiles[1][:, 512:], in_=qT[1][:, 512:])
            nc.sync.dma_start(out=kv_sb[0][0][:, 512:], in_=kT[0][:, 512:])
            nc.scalar.dma_start(
                out=kv_sb[0][1].rearrange("p (t c) -> p t c", t=NT)[:, 0:2, :],
                in_=v1[0].rearrange("t p c -> p t c")[:, 0:2, :],
            )
            nc.scalar.dma_start(
                out=kv_sb[0][1].rearrange("p (t c) -> p t c", t=NT)[:, 2:, :],
                in_=v1[0].rearrange("t p c -> p t c")[:, 2:, :],
            )
            q_tiles[2] = qpool.tile([128, S], BF, tag="q", name="q_sb2")
            q_tiles[3] = qpool.tile([128, S], BF, tag="q", name="q_sb3")
            nc.sync.dma_start(out=q_tiles[2][:, 0:512], in_=qT[2][:, 0:512])
            nc.sync.dma_start(out=q_tiles[3][:, 0:512], in_=qT[3][:, 0:512])
            nc.sync.dma_start(out=q_tiles[2][:, 512:], in_=qT[2][:, 512:])
            nc.sync.dma_start(out=q_tiles[3][:, 512:], in_=qT[3][:, 512:])
            load_kv(1)

            for h0 in range(0, H, 2):
                hs = (h0, h0 + 1)
                last = h0 == H - 2
                kvh = h0 // G
                kT_sb, v1_sb = kv_sb[kvh]
                load_q(h0 + 2)
                load_q(h0 + 3)
                if h0 % G == 0 and kvh + 2 < KVH:
                    load_kv(kvh + 2)
                p_loc = {}  # (h, qc, kt) -> (tile, off, qoff) for off-diag
                d_loc = {}  # (h, qc, kt) -> d-tile slice for the diagonal
                po2 = {}    # (h, qt//2) -> psum tile [128, 258]
                osb = {
                    h: opool.tile([128, 1032], BF, tag="o", name=f"o_{h}")
                    for h in hs
                }

                def pv_run(h, qt, start_kt=0, stop_kt=None):
                    # accumulate P.T @ [V|1] over qt's k tiles back-to-back;
                    # two q-tiles share one psum bank (single start=True per
                    # bank)
                    qc = qt // 4
                    if qt % 2 == 0 and start_kt == 0:
                        po2[(h, qt // 2)] = popool.tile(
                            [128, 258], F32, tag="po", name=f"po_{h}_{qt}"
                        )
                    po = po2[(h, qt // 2)]
                    base = (qt % 2) * 129
                    end_kt = qt + 1 if stop_kt is None else stop_kt
                    for kt in range(start_kt, end_kt):
                        if kt == qt:
                            lhsT = d_loc[(h, qc, kt)]
                        else:
                            t, off, qoff = p_loc[(h, qc, kt)]
                            j = qt - qc * 4
                            lhsT = t[:, off + (j - qoff) * 128:
                                     off + (j - qoff) * 128 + 128]
                        nc.tensor.matmul(
                            po[:, base: base + 129],
                            lhsT=lhsT,
                            rhs=v1_sb[:, kt * VW: kt * VW + 129],
                            start=(kt == 0 and qt % 2 == 0 and start_kt == 0),
                            stop=(kt == qt),
                            skip_group_check=True,
                        )

                def evac(h, pr):
                    # raw bf16 copy of the po pair (out cols + rowsum);
                    # host does the softmax division
                    nc.scalar.copy(
                        osb[h][:, pr * 258: pr * 258 + 258], po2[(h, pr)]
                    )

                pending = []
                for gi, (qc, tw, entries) in enumerate(GROUPS6):
                    ps_t = {}
                    for h in hs:
                        ps = pspool.tile(
                            [128, 1024], F32, tag="ps",
                            name=f"ps_{h}_{gi}",
                        )
                        ps_t[h] = ps
                        for kt, qoff, w, off, isdiag in entries:
                            nc.tensor.matmul(
                                ps[:, off: off + w],
                                lhsT=kT_sb[:, kt * 128: kt * 128 + 128],
                                rhs=q_tiles[h][
                                    :, qc * 512 + qoff * 128:
                                    qc * 512 + qoff * 128 + w
                                ],
                                start=True, stop=True, skip_group_check=True,
                            )
                    # last unit: pre-accumulate qt6 (kt0-5, opens the bank
                    # with start=True) then qt7 (kt0-5) during G5's QK/exp
                    # so only kt6/kt7 matmuls trail the final exp
                    if last and gi == 5:
                        for h in hs:
                            pv_run(h, 6, stop_kt=6)
                            pv_run(h, 7, stop_kt=6)
                    for h in hs:
                        if (gi, h - h0) in SCHRAU:
                            # bf16-Schraudolph exp on the DVE offloads ACT:
                            # bits = rne(s*128*log2e + (127*128 - 7.4)),
                            # bitcast bf16 (rel err ~1.8% RMS; largely
                            # cancels in softmax num/denom)
                            p_i16 = ppool.tile(
                                [128, tw], I16, tag="p", name=f"p_{h}_{gi}"
                            )
                            nc.vector.tensor_scalar(
                                p_i16, ps_t[h][:, 0:tw],
                                184.6644353, 16248.6,
                                mybir.AluOpType.mult, mybir.AluOpType.add,
                            )
                            p_sb = p_i16.bitcast(BF)
                        else:
                            p_sb = ppool.tile(
                                [128, tw], BF, tag="p", name=f"p_{h}_{gi}"
                            )
                            nc.scalar.activation(
                                p_sb, ps_t[h][:, 0:tw],
                                mybir.ActivationFunctionType.Exp,
                            )
                        diags = [e for e in entries if e[4]]
                        for kt, qoff, w, off, isdiag in entries:
                            if not isdiag:
                                p_loc[(h, qc, kt)] = (p_sb, off, qoff)
                        if diags:
                            # both diagonal subtiles (psum cols 0:256) in
                            # one DVE multiply vs the broadcast tri mask
                            d2 = dpool.tile(
                                [128, 256], BF, tag="d", name=f"d_{h}_{gi}"
                            )
                            nc.vector.tensor_mul(
                                d2.rearrange("p (a b) -> p a b", a=2),
                                p_sb[:, 0:256].rearrange(
                                    "p (a b) -> p a b", a=2
                                ),
                                mask_sb.rearrange(
                                    "p (a b) -> p a b", a=1
                                ).broadcast_to([128, 2, 128]),
                            )
                            for di, (kt, qoff, w, off, isdiag) in enumerate(
                                diags
                            ):
                                d_loc[(h, qc, kt)] = d2[
                                    :, di * 128: di * 128 + 128
                                ]
                    # emit PV runs one group late so the next group's QK +
                    # exp stay ahead of the PV burst on the PE stream
                    for h3, qt3, s3 in pending:
                        pv_run(h3, qt3, start_kt=s3)
                        if qt3 % 2 == 1:
                            evac(h3, qt3 // 2)
                    pending = []
                    if gi in PV_AFTER:
                        for h in hs:
                            for qt3 in PV_AFTER[gi]:
                                pending.append(
                                    (h, qt3, 6 if (last and qt3 >= 6) else 0)
                                )
                    if last and gi >= 4:
                        # tail: run PVs eagerly, per-pair store right after
                        for h3, qt3, s3 in pending:
                            pv_run(h3, qt3, start_kt=s3)
                            if qt3 % 2 == 1:
                                pr = qt3 // 2
                                evac(h3, pr)
                                ring = nc.sync if qt3 == 5 else nc.gpsimd
                                ring.dma_start(
                                    out=out[h3, pr],
                                    in_=osb[h3][:, pr * 258: pr * 258 + 258],
                                )
                        pending = []
                if not last:
                    for h3, qt3, s3 in pending:
                        pv_run(h3, qt3, start_kt=s3)
                        if qt3 % 2 == 1:
                            evac(h3, qt3 // 2)
                    # one batched store per head on the sync HWDGE ring
                    for h in hs:
                        nc.sync.dma_start(
                            out=out[h].rearrange("a p b -> p a b"),
                            in_=osb[h].rearrange("p (a b) -> p a b", a=4),
                        )
                else:
                    # store the first two pairs of each last-unit head
                    for h in hs:
                        nc.sync.dma_start(
                            out=out[h, 0:2].rearrange("a p b -> p a b"),
                            in_=osb[h][:, 0:516].rearrange(
                                "p (a b) -> p a b", a=2
                            ),
                        )

    nc.compile()
    return nc


def _get_nc():
    global _NC
    if _NC is None:
        _NC = _build_nc()
    return _NC


def make_in_maps(q, k, v, k_cache, v_cache, slot_mapping, block_tables):
    nb, bs, kvh, d = k_cache.shape
    # store_kvcache scatter (mirrors reference semantics on host)
    kc = k_cache.reshape(nb * bs, kvh, d).copy()
    vc = v_cache.reshape(nb * bs, kvh, d).copy()
    kc[slot_mapping] = k
    vc[slot_mapping] = v
    b, mb = block_tables.shape
    s = q.shape[0] // b
    pos = np.arange(s)
    slot_grid = block_tables[:, pos // bs] * bs + (pos % bs)  # [B, S]
    kf = kc[slot_grid]  # [B, S, KVH, D]
    vf = vc[slot_grid]
    qb = q.reshape(b, s, H, D)

    bf16 = ml_dtypes.bfloat16
    in_maps = []
    for i in range(b):
        qTi = np.ascontiguousarray(
            qb[i].transpose(1, 2, 0) * np.float32(SCALE)
        ).astype(bf16)
        kTi = np.ascontiguousarray(kf[i].transpose(1, 2, 0)).astype(bf16)
        vh = vf[i].transpose(1, 0, 2).reshape(KVH, NT, 128, D)
        v1i = np.zeros((KVH, NT, 128, VW), dtype=bf16)
        v1i[..., :D] = vh.astype(bf16)
        v1i[..., D] = 1.0
        in_maps.append({"qT": qTi, "kT": kTi, "v1": v1i})
    return in_maps


def kernel(q, k, v, k_cache, v_cache, slot_mapping, block_tables):
    # accept jax or numpy inputs
    q = np.asarray(q)
    k = np.asarray(k)
    v = np.asarray(v)
    k_cache = np.asarray(k_cache)
    v_cache = np.asarray(v_cache)
    slot_mapping = np.asarray(slot_mapping)
    block_tables = np.asarray(block_tables)
    out_dtype = q.dtype
    in_maps = make_in_maps(q, k, v, k_cache, v_cache, slot_mapping, block_tables)
    nc = _get_nc()
    res = run_bass_kernel_spmd(nc, in_maps, core_ids=list(range(8)))
    outs = []
    for i in range(B):
        o4 = np.asarray(res.results[i]["out"]).astype(np.float32)
        # [H, 4 pairs, 128 rows, 2*129] -> softmax division on host
        arr = o4.reshape(H, 4, 128, 2, 129)
        o = arr[..., :128] / arr[..., 128:129]  # [H, 4, 128, 2, 128]
        o = o.transpose(0, 1, 3, 2, 4).reshape(H, S, D)
        outs.append(o.transpose(1, 0, 2))  # [S, H, D]
    return np.concatenate(outs, axis=0).astype(out_dtype)


# revision 3
# speedup vs baseline: 1.1075x; 1.1075x over previous
"""Paged-KV-cache causal GQA attention on 8 TRN2 NeuronCores.

Problem shape (hardcoded): B=8 seqs x S=1024 tokens, H=32 q-heads,
KVH=8 kv-heads (GQA group 4), D=128, block_size=256, 40 cache blocks.

Sharding: data parallel, one sequence per core. Host does the
store_kvcache scatter + block-table gather (layout work) and per-core
layout prep (head-major transposes + bf16 cast, scale folded into q);
each core runs causal flash attention for its sequence over all 32
heads. Softmax denominator division happens on the host: the device
ships unnormalized PV output plus the rowsum column.

Device algorithm per head, two heads interleaved, 6 score groups per
head (each <=1024 psum cols = 2 banks; pspool 3-deep so QK rarely
waits on exp):
  warmup: 32 dummy matmuls bridge the initial DMA wait so the PE HAM
          clock gate stays 8/8 into the first real matmul (~15us).
  QK:     scores^T[k,q] = K^T.T @ Q^T (PE, bf16) into a [128,<=1024]
          psum tile, one matmul per k-tile (LDWEIGHTS hides under the
          previous matmul's >=384-col stream).
  exp:    one wide op per (head, group); ACT carries most of it. One
          1024-col group per pair runs a bf16-Schraudolph exp on the
          DVE (bits = rne(s*184.664 + 16248.6) as int16, bitcast bf16;
          rel err ~1.8% RMS, largely cancels in softmax num/denom).
  mask:   diagonal tiles masked into separate [128,128] tiles (DVE).
  PV:     po[q,0:129] += P^T.T @ [V|1] per (q-tile, k-tile), two
          q-tiles per psum bank, emitted one group late so QK + exp
          stay ahead of the PV burst on the PE stream.
  evac:   DVE copies the raw [128,258] po pair (128 out cols + rowsum
          col per q-tile) to SBUF as bf16 (no reciprocal/normalize on
          device); one store per head on the sync HWDGE ring
          (scalar-ring stores would queue behind exps).
Last unit runs PVs eagerly and pre-accumulates qt6/qt7 over kt0-5
during the final group so only 3 matmuls + 2 copies trail the last
exp.
"""

import sys

import numpy as np
import ml_dtypes

sys.path.insert(0, "/opt/trn_rl_repo")

import concourse.bass as bass  # noqa: E402
import concourse.mybir as mybir  # noqa: E402
import concourse.tile as tile  # noqa: E402
from concourse import bacc  # noqa: E402
from concourse.bass_utils import run_bass_kernel_spmd  # noqa: E402

B, S = 8, 1024
H, KVH, D = 32, 8, 128
G = H // KVH
NT = S // 128  # 8 k/q tiles of 128 per sequence
VW = 132  # v tile row: 128 v cols + ones col + pad
SCALE = 1.0 / float(np.sqrt(D))
BF = mybir.dt.bfloat16
F32 = mybir.dt.float32
I16 = mybir.dt.int16
_NC = None

# Score groups: (qc, tw, entries) with entries = ordered matmuls
# (kt, q_tile_off_in_chunk, width, psum_off). No region crosses a
# 512-f32 psum bank boundary.
GROUPS6 = [
    (0, 896, [(0, 0, 512, 0), (1, 1, 384, 512)]),
    (0, 384, [(2, 2, 256, 0), (3, 3, 128, 256)]),
    (1, 1024, [(0, 0, 512, 0), (1, 0, 512, 512)]),
    (1, 1024, [(2, 0, 512, 0), (3, 0, 512, 512)]),
    (1, 896, [(4, 0, 512, 0), (5, 1, 384, 512)]),
    (1, 384, [(6, 2, 256, 0), (7, 3, 128, 256)]),
]
# q-tile pairs emitted (one group late) after each group index
PV_AFTER = {0: (0, 1), 1: (2, 3), 4: (4, 5), 5: (6, 7)}
# groups whose exp runs as bf16-Schraudolph on the DVE: (gi, head-in-pair)
SCHRAU = {(2, 0)}


def _build_nc():
    nc = bacc.Bacc("TRN2", target_bir_lowering=False, debug=False, num_devices=8)
    qT = nc.dram_tensor("qT", [H, D, S], BF, kind="ExternalInput").ap()
    kT = nc.dram_tensor("kT", [KVH, D, S], BF, kind="ExternalInput").ap()
    v1 = nc.dram_tensor("v1", [KVH, NT, 128, VW], BF, kind="ExternalInput").ap()
    # per head: 4 q-tile pairs x [128 rows, 2*(128 out + rowsum)]
    out = nc.dram_tensor("out", [H, 4, 128, 258], BF, kind="ExternalOutput").ap()
    mask_np = np.triu(np.ones((128, 128), dtype=ml_dtypes.bfloat16))
    mask_dram = nc.inline_tensor(mask_np, "tri_mask").ap()

    with tile.TileContext(nc) as tc:
        with (
            tc.tile_pool(name="singles", bufs=1) as singles,
            tc.tile_pool(name="qpool", bufs=6) as qpool,
            tc.tile_pool(name="ppool", bufs=16) as ppool,
            tc.tile_pool(name="dpool", bufs=22) as dpool,
            tc.tile_pool(name="opool", bufs=4) as opool,
            tc.tile_pool(name="pspool", bufs=3, space="PSUM") as pspool,
            tc.tile_pool(name="popool", bufs=2, space="PSUM") as popool,
        ):
            # --- HAM warmup: dummy matmuls with no data deps keep the
            # PE busy until the first real QK data lands (~15us) so the
            # clock gate is 8/8 when real work starts ---
            warm_sb = singles.tile([128, 256], BF, name="warm_sb")
            nc.vector.memset(warm_sb, 0.0)
            dummy_ps = popool.tile([128, 258], F32, tag="po", name="dummy_ps")
            for i in range(20):
                nc.tensor.matmul(
                    dummy_ps[:, 0:256], lhsT=warm_sb[:, 0:128], rhs=warm_sb,
                    start=True, stop=True, skip_group_check=True,
                )
            for i in range(12):
                nc.tensor.matmul(
                    dummy_ps[:, 0:128], lhsT=warm_sb[:, 0:128],
                    rhs=warm_sb[:, 0:128],
                    start=True, stop=True, skip_group_check=True,
                )

            mask_sb = singles.tile([128, 128], BF)
            kv_sb = []
            for kvh in range(KVH):
                k_t = singles.tile([128, S], BF, name=f"kT_sb{kvh}", tag=f"kT{kvh}")
                v_t = singles.tile(
                    [128, NT * VW], BF, name=f"v1_sb{kvh}", tag=f"v1{kvh}"
                )
                kv_sb.append((k_t, v_t))

            def load_kv(kvh):
                # kT on the sync HWDGE ring; v1 on the gpsimd SWDGE ring so
                # the two streams' kickoffs and transfers run in parallel
                nc.sync.dma_start(out=kv_sb[kvh][0], in_=kT[kvh])
                nc.gpsimd.dma_start(
                    out=kv_sb[kvh][1].rearrange("p (t c) -> p t c", t=NT),
                    in_=v1[kvh].rearrange("t p c -> p t c"),
                )

            q_tiles = {}

            def load_q(h):
                if h < H and h not in q_tiles:
                    q_tiles[h] = qpool.tile([128, S], BF, tag="q", name=f"q_sb{h}")
                    nc.sync.dma_start(out=q_tiles[h], in_=qT[h])

            # fast start: the head phase is HBM-bandwidth-bound (all 8
            # cores burst-load at once), so critical bytes go on ONE ring
            # in strict need order (first group needs only kT cols 0:256);
            # v1[0] (needed ~2us later for the first PVs) rides the
            # otherwise-idle scalar HWDGE ring
            q_tiles[0] = qpool.tile([128, S], BF, tag="q", name="q_sb0")
            q_tiles[1] = qpool.tile([128, S], BF, tag="q", name="q_sb1")
            nc.sync.dma_start(out=kv_sb[0][0][:, 0:256], in_=kT[0][:, 0:256])
            nc.sync.dma_start(out=q_tiles[0][:, 0:512], in_=qT[0][:, 0:512])
            nc.sync.dma_start(out=mask_sb, in_=mask_dram)
            nc.sync.dma_start(out=kv_sb[0][0][:, 256:512], in_=kT[0][:, 256:512])
            nc.sync.dma_start(out=q_tiles[1][:, 0:512], in_=qT[1][:, 0:512])
            nc.sync.dma_start(out=q_tiles[0][:, 512:], in_=qT[0][:, 512:])
            nc.sync.dma_start(out=q_tiles[1][:, 512:], in_=qT[1][:, 512:])
            nc.sync.dma_start(out=kv_sb[0][0][:, 512:], in_=kT[0][:, 512:])
            nc.scalar.dma_start(
                out=kv_sb[0][1].rearrange("p (t c) -> p t c", t=NT)[:, 0:2, :],
                in_=v1[0].rearrange("t p c -> p t c")[:, 0:2, :],
            )
            nc.scalar.dma_start(
                out=kv_sb[0][1].rearrange("p (t c) -> p t c", t=NT)[:, 2:, :],
                in_=v1[0].rearrange("t p c -> p t c")[:, 2:, :],
            )
            q_tiles[2] = qpool.tile([128, S], BF, tag="q", name="q_sb2")
            q_tiles[3] = qpool.tile([128, S], BF, tag="q", name="q_sb3")
            nc.sync.dma_start(out=q_tiles[2][:, 0:512], in_=qT[2][:, 0:512])
            nc.sync.dma_start(out=q_tiles[3][:, 0:512], in_=qT[3][:, 0:512])
            nc.sync.dma_start(out=q_tiles[2][:, 512:], in_=qT[2][:, 512:])
            nc.sync.dma_start(out=q_tiles[3][:, 512:], in_=qT[3][:, 512:])
            load_kv(1)

            for h0 in range(0, H, 2):
                hs = (h0, h0 + 1)
                last = h0 == H - 2
                kvh = h0 // G
                kT_sb, v1_sb = kv_sb[kvh]
                load_q(h0 + 2)
                load_q(h0 + 3)
                if h0 % G == 0 and kvh + 2 < KVH:
                    load_kv(kvh + 2)
                p_loc = {}  # (h, qc, kt) -> (tile, off, qoff)
                d_loc = {}  # (h, qc, kt) -> masked diagonal tile
                po2 = {}    # (h, qt//2) -> psum tile [128, 258]
                osb = {
                    h: opool.tile([128, 1032], BF, tag="o", name=f"o_{h}")
                    for h in hs
                }

                def pv_run(h, qt, start_kt=0, stop_kt=None):
                    # accumulate P.T @ [V|1] over qt's k tiles back-to-back;
                    # two q-tiles share one psum bank (single start=True per
                    # bank)
                    qc = qt // 4
                    if qt % 2 == 0 and start_kt == 0:
                        po2[(h, qt // 2)] = popool.tile(
                            [128, 258], F32, tag="po", name=f"po_{h}_{qt}"
                        )
                    po = po2[(h, qt // 2)]
                    base = (qt % 2) * 129
                    end_kt = qt + 1 if stop_kt is None else stop_kt
                    for kt in range(start_kt, end_kt):
                        if kt == qt:
                            lhsT = d_loc[(h, qc, kt)]
                        else:
                            t, off, qoff = p_loc[(h, qc, kt)]
                            j = qt - qc * 4
                            lhsT = t[:, off + (j - qoff) * 128:
                                     off + (j - qoff) * 128 + 128]
                        nc.tensor.matmul(
                            po[:, base: base + 129],
                            lhsT=lhsT,
                            rhs=v1_sb[:, kt * VW: kt * VW + 129],
                            start=(kt == 0 and qt % 2 == 0 and start_kt == 0),
                            stop=(kt == qt),
                            skip_group_check=True,
                        )

                def evac(h, pr):
                    # raw bf16 copy of the po pair (out cols + rowsum);
                    # host does the softmax division
                    nc.vector.tensor_copy(
                        osb[h][:, pr * 258: pr * 258 + 258], po2[(h, pr)]
                    )

                pending = []
                for gi, (qc, tw, entries) in enumerate(GROUPS6):
                    ps_t = {}
                    for h in hs:
                        ps = pspool.tile(
                            [128, 1024], F32, tag="ps", name=f"ps_{h}_{gi}",
                        )
                        ps_t[h] = ps
                        for kt, qoff, w, off in entries:
                            nc.tensor.matmul(
                                ps[:, off: off + w],
                                lhsT=kT_sb[:, kt * 128: kt * 128 + 128],
                                rhs=q_tiles[h][
                                    :, qc * 512 + qoff * 128:
                                    qc * 512 + qoff * 128 + w
                                ],
                                start=True, stop=True, skip_group_check=True,
                            )
                    # last unit: pre-accumulate qt6 (kt0-5, opens the bank
                    # with start=True) then qt7 (kt0-5) during G5's QK/exp
                    # so only kt6/kt7 matmuls trail the final exp
                    if last and gi == 5:
                        for h in hs:
                            pv_run(h, 6, stop_kt=6)
                            pv_run(h, 7, stop_kt=6)
                    for h in hs:
                        if (gi, h - h0) in SCHRAU:
                            # bf16-Schraudolph exp on the DVE offloads ACT
                            p_i16 = ppool.tile(
                                [128, tw], I16, tag="p", name=f"p_{h}_{gi}"
                            )
                            nc.vector.tensor_scalar(
                                p_i16, ps_t[h][:, 0:tw],
                                184.6644353, 16248.6,
                                mybir.AluOpType.mult, mybir.AluOpType.add,
                            )
                            p_sb = p_i16.bitcast(BF)
                        else:
                            p_sb = ppool.tile(
                                [128, tw], BF, tag="p", name=f"p_{h}_{gi}"
                            )
                            nc.scalar.activation(
                                p_sb, ps_t[h][:, 0:tw],
                                mybir.ActivationFunctionType.Exp,
                            )
                        for kt, qoff, w, off in entries:
                            p_loc[(h, qc, kt)] = (p_sb, off, qoff)
                            if kt >= qc * 4:  # diagonal: upper-tri mask
                                dt_ = dpool.tile(
                                    [128, 128], BF, tag="d",
                                    name=f"d_{h}_{gi}_{kt}",
                                )
                                j = kt - qc * 4
                                nc.vector.tensor_mul(
                                    dt_,
                                    p_sb[:, off + (j - qoff) * 128:
                                         off + (j - qoff) * 128 + 128],
                                    mask_sb,
                                )
                                d_loc[(h, qc, kt)] = dt_
                    # emit PV runs one group late so the next group's QK +
                    # exp stay ahead of the PV burst on the PE stream
                    for h3, qt3, s3 in pending:
                        pv_run(h3, qt3, start_kt=s3)
                        if qt3 % 2 == 1:
                            evac(h3, qt3 // 2)
                    pending = []
                    if gi in PV_AFTER:
                        for h in hs:
                            for qt3 in PV_AFTER[gi]:
                                pending.append(
                                    (h, qt3, 6 if (last and qt3 >= 6) else 0)
                                )
                    if last and gi >= 4:
                        # tail: run PVs eagerly, per-pair store right after
                        for h3, qt3, s3 in pending:
                            pv_run(h3, qt3, start_kt=s3)
                            if qt3 % 2 == 1:
                                pr = qt3 // 2
                                evac(h3, pr)
                                ring = nc.sync if qt3 == 5 else nc.gpsimd
                                ring.dma_start(
                                    out=out[h3, pr],
                                    in_=osb[h3][:, pr * 258: pr * 258 + 258],
                                )
                        pending = []
                if not last:
                    for h3, qt3, s3 in pending:
                        pv_run(h3, qt3, start_kt=s3)
                        if qt3 % 2 == 1:
                            evac(h3, qt3 // 2)
                    # one batched store per head on the sync HWDGE ring
                    for h in hs:
                        nc.sync.dma_start(
                            out=out[h].rearrange("a p b -> p a b"),
                            in_=osb[h].rearrange("p (a b) -> p a b", a=4),
                        )
                else:
                    # store the first two pairs of each last-unit head
                    for h in hs:
                        nc.sync.dma_start(
                            out=out[h, 0:2].rearrange("a p b -> p a b"),
                            in_=osb[h][:, 0:516].rearrange(
                                "p (a b) -> p a b", a=2
                            ),
                        )

    nc.compile()
    return nc


def _get_nc():
    global _NC
    if _NC is None:
        _NC = _build_nc()
    return _NC


def make_in_maps(q, k, v, k_cache, v_cache, slot_mapping, block_tables):
    nb, bs, kvh, d = k_cache.shape
    # store_kvcache scatter (mirrors reference semantics on host)
    kc = k_cache.reshape(nb * bs, kvh, d).copy()
    vc = v_cache.reshape(nb * bs, kvh, d).copy()
    kc[slot_mapping] = k
    vc[slot_mapping] = v
    b, mb = block_tables.shape
    s = q.shape[0] // b
    pos = np.arange(s)
    slot_grid = block_tables[:, pos // bs] * bs + (pos % bs)  # [B, S]
    kf = kc[slot_grid]  # [B, S, KVH, D]
    vf = vc[slot_grid]
    qb = q.reshape(b, s, H, D)

    bf16 = ml_dtypes.bfloat16
    in_maps = []
    for i in range(b):
        qTi = np.ascontiguousarray(
            qb[i].transpose(1, 2, 0) * np.float32(SCALE)
        ).astype(bf16)
        kTi = np.ascontiguousarray(kf[i].transpose(1, 2, 0)).astype(bf16)
        vh = vf[i].transpose(1, 0, 2).reshape(KVH, NT, 128, D)
        v1i = np.zeros((KVH, NT, 128, VW), dtype=bf16)
        v1i[..., :D] = vh.astype(bf16)
        v1i[..., D] = 1.0
        in_maps.append({"qT": qTi, "kT": kTi, "v1": v1i})
    return in_maps


def kernel(q, k, v, k_cache, v_cache, slot_mapping, block_tables):
    # accept jax or numpy inputs
    q = np.asarray(q)
    k = np.asarray(k)
    v = np.asarray(v)
    k_cache = np.asarray(k_cache)
    v_cache = np.asarray(v_cache)
    slot_mapping = np.asarray(slot_mapping)
    block_tables = np.asarray(block_tables)
    out_dtype = q.dtype
    in_maps = make_in_maps(q, k, v, k_cache, v_cache, slot_mapping, block_tables)
    nc = _get_nc()
    res = run_bass_kernel_spmd(nc, in_maps, core_ids=list(range(8)))
    outs = []
    for i in range(B):
        o4 = np.asarray(res.results[i]["out"]).astype(np.float32)
        # [H, 4 pairs, 128 rows, 2*129] -> softmax division on host
        arr = o4.reshape(H, 4, 128, 2, 129)
        o = arr[..., :128] / arr[..., 128:129]  # [H, 4, 128, 2, 128]
        o = o.transpose(0, 1, 3, 2, 4).reshape(H, S, D)
        outs.append(o.transpose(1, 0, 2))  # [S, H, D]
    return np.concatenate(outs, axis=0).astype(out_dtype)
